# revision 1
# baseline (speedup 1.0000x reference)
"""Trainium2 Bass kernel v2 for nn_MultiHeadAttention_85925115723936.

Contract: kernel(**inputs) takes the FULL unsharded inputs from
setup_inputs() (x [16,1024,1024] f32, Wq/Wk/Wv [1024,64], Wp [1024,1024],
bp [1024]) and returns the FULL [16, 1024, 1024] float32 output.

Sharding: data-parallel over batch — 16 batches across 8 NeuronCores
(2 per core), zero cross-core communication.

All H=16 heads share one weight set, so the H-way concat collapses into a
folded projection (wp_eff = sum_h Wp[h*hs:(h+1)*hs, :]), bias folded as an
extra row driven by the softmax denominator; only the causal triangle of
scores is computed; the denominator falls out of a ones-column in v_aug.

x is loaded as float8_e3m4 (e3m4's 4 mantissa bits keep l2 error at
~1.4% vs the 2e-2 gate while halving input DMA; the PE consumes mixed
fp8/bf16 operands directly).

v2 structural changes vs v1:
- PE warmup matmuls on a zeroed scratch tile cover the initial x-load fill
  so the PE p-state ramp (0.65->1.2->2.4GHz after 3us continuous busy)
  completes before real work starts, and the PE never idles at the head.
- All constants arrive in ONE packed [128, 2753] DMA (HWDGE descriptor
  generation is a serialized ~630ns/DMA shared resource): wqk | wv | wp_aug
  | mask | idb | unit.
- PSUM is managed as a single rotating ring of 1-bank [128,512]-shaped
  tiles (bufs=7) + 1 small bank, so qk/v/scores/nd/out all timeshare the
  8 banks without static carve-out conflicts.
- Scores matmuls are causally trimmed (only t >= s-chunk start computed).
- Evictions/normalizations are spread across ACT/DVE/Pool; exp stays on ACT.
- Batch b+1's qk/v matmuls are interleaved into batch b's nd/out phase to
  keep the PE gapless.
"""

import numpy as np
import ml_dtypes

import concourse.mybir as mybir
import concourse.tile as tile
from concourse import bacc
from concourse.bass_utils import run_bass_kernel_spmd

BF_NP = ml_dtypes.bfloat16
BF = mybir.dt.bfloat16
F32 = mybir.dt.float32
EXP = mybir.ActivationFunctionType.Exp
COPY = mybir.ActivationFunctionType.Copy

B, T, D, H, HS = 16, 1024, 1024, 16, 64
NCORES = 8
NB = B // NCORES     # batches per core
NCH = 8              # 1024 / 128 chunks
OUT_DT = BF

# packed-constant free-dim offsets (bf16 elems per partition)
OFF_WQK = 0          # [128, 8, 128] -> 1024
OFF_WV = 1024        # [128, 8, 64]  -> 512
OFF_WP = 1536        # [65, 1024] on partitions 0:65
OFF_MASK = 2560      # [128, 128]
OFF_IDB = 2688       # [64, 64] on partitions 0:64
OFF_UNIT = 2752      # [65, 1] on partitions 0:65
CONST_W = 2753

N_WARMUP = 0
X_DT = mybir.dt.float8e3
X_DT_NP = ml_dtypes.float8_e3m4



def _build_nc(n_reps: int = 1, n_cores: int = NCORES):
    # Warmup matmuls ramp the PE p-state at cold start; in an n_reps>1
    # timing loop the PE is already hot from the previous iteration, so
    # they would be pure per-iteration overhead.
    warmup = N_WARMUP > 0
    nc = bacc.Bacc("TRN2", target_bir_lowering=False, debug=False, num_devices=n_cores)

    xT_d = nc.dram_tensor("xT", [NB, D, T], X_DT, kind="ExternalInput")
    cpack_d = nc.dram_tensor("cpack", [128, CONST_W], BF, kind="ExternalInput")
    out_d = nc.dram_tensor("out", [NB, T, D], OUT_DT, kind="ExternalOutput")

    with tile.TileContext(nc) as tc:
        with (
            tc.tile_pool(name="const", bufs=1) as cpool,
            tc.tile_pool(name="sb", bufs=2) as sb,
            tc.tile_pool(name="ps", bufs=6, space="PSUM") as ps,
            tc.tile_pool(name="pss", bufs=2, space="PSUM") as pss,
        ):
            # ---- constants: one packed DMA on the ACT ring; wqk columns
            # first so the first qk matmul's weights land earliest ----
            cs = cpool.tile([128, CONST_W], BF, tag="cpack")
            nc.scalar.dma_start(out=cs[:, 0:1024], in_=cpack_d.ap()[:, 0:1024])

            def wqk(c):
                return cs[:, OFF_WQK + c * 128 : OFF_WQK + (c + 1) * 128]

            def wv(c):
                return cs[:, OFF_WV + c * 64 : OFF_WV + (c + 1) * 64]

            wp = cs[0:65, OFF_WP : OFF_WP + 1024]
            mask = cs[:, OFF_MASK : OFF_MASK + 128]
            idb = cs[0:64, OFF_IDB : OFF_IDB + 64]
            unit = cs[0:65, OFF_UNIT : OFF_UNIT + 1]

            # ---- warmup scratch (zeroed so CoreSim finiteness holds) ----
            scratch = cpool.tile([128, 512], BF, tag="scratch")
            nc.vector.memset(scratch[:], 0.0)

            # ---- x loads: all emitted upfront on the SP ring; batch0 in
            # fine grains so qk/v are never head-of-line blocked ----
            xs = []
            for b in range(NB):
                xt = sb.tile([128, NCH, T], X_DT, tag=f"x{b}", bufs=1, name=f"x{b}")
                xs.append(xt)

            def xload(b, pieces):
                xr = xT_d.ap()[b].rearrange("(c p) t -> p c t", p=128)
                for c0, c1 in pieces:
                    nc.sync.dma_start(out=xs[b][:, c0:c1, :], in_=xr[:, c0:c1, :])

            # cpack part 2 (wv/wp/mask/...) emitted directly after part 1;
            # x loads are emitted per iteration inside run_once
            nc.scalar.dma_start(
                out=cs[:, 1024:CONST_W], in_=cpack_d.ap()[:, 1024:CONST_W]
            )

            # ---- warmup matmuls: fill the DMA window, ramp the PE clock ----
            def emit_warmups(n):
                for w in range(n):
                    wu = pss.tile([128, 512], F32, tag="small", name="wu")
                    nc.tensor.matmul(wu[:], scratch[:, 0:128], scratch[:],
                                     start=True, stop=True)

            # ---------------- per-batch emission pieces ----------------
            # Each batch body is split into closures so batch b+1's qk/v can
            # be interleaved into batch b's nd/out phase.

            def make_batch(b):
                st = {}
                xt = xs[b]

                def xsl(c, h):
                    return xt[:, c, h * 512 : (h + 1) * 512]

                def qk_half(h):
                    qk_ps = ps.tile([128, 512], F32, tag="big")
                    for c in range(NCH):
                        nc.tensor.matmul(
                            qk_ps[:],
                            wqk(c),
                            xsl(c, h),
                            start=(c == 0),
                            stop=(c == NCH - 1),
                        )
                    st[f"qk{h}"] = qk_ps

                def qk_evict(h):
                    if "qT" not in st:
                        st["qT"] = sb.tile([64, T], BF, tag="qT", name="qT")
                        st["kT"] = sb.tile([64, T], BF, tag="kT", name="kT")
                    qp = st[f"qk{h}"]
                    nc.vector.tensor_copy(
                        st["qT"][:, h * 512 : (h + 1) * 512], qp[0:64, :]
                    )
                    nc.vector.tensor_copy(
                        st["kT"][:, h * 512 : (h + 1) * 512], qp[64:128, :]
                    )

                def v_chain_start(t_, c):
                    # direct-layout v for t-chunk t_: accumulate over
                    # d-chunks with x as the stationary operand -> v lands
                    # as [t, h] (v_aug orientation), no transposes. One
                    # pending accumulation chain per PSUM bank, so each
                    # chain gets its own small tile (2 slots ping-pong).
                    key = f"v_ps{t_}"
                    if key not in st:
                        pool_ = pss if t_ in (0, 1, 6, 7) else ps
                        tag_ = "small" if t_ in (0, 1, 6, 7) else "big"
                        st[key] = pool_.tile([128, HS], F32, tag=tag_,
                                             name=key)
                    nc.tensor.matmul(
                        st[key][:],
                        xt[:, c, t_ * 128 : (t_ + 1) * 128],
                        wv(c),
                        start=(c == 0),
                        stop=(c == NCH - 1),
                    )

                def v_chain_evict(t_):
                    nc.vector.tensor_copy(
                        st["v_aug"][:, t_, 0:HS], st[f"v_ps{t_}"][:]
                    )
                    del st[f"v_ps{t_}"]

                def sc_init():
                    st["attnT"] = sb.tile([128, NCH, T], BF, tag="attnT", name="attnT")
                    st["v_aug"] = sb.tile([128, NCH, HS + 1], BF, tag="v_aug", name="v_aug")
                    nc.gpsimd.memset(st["v_aug"][:, :, HS], 1.0)
                    st["ndT"] = sb.tile([65, NCH, 128], BF, tag="ndT", name="ndT")
                    st["recip"] = sb.tile([128, NCH], F32, tag="recip", name="recip")
                    st["out_sb"] = sb.tile([128, NCH, D], OUT_DT, tag="out_sb", name="out_sb")

                def sc_piece(i, h):
                    # scores^T for s-chunk i, t in [max(128i, 512h), 512(h+1))
                    t0 = max(i * 128, h * 512)
                    t1 = (h + 1) * 512
                    if t0 >= t1:
                        return
                    sc_ps = ps.tile([128, 512], F32, tag="big")
                    nc.tensor.matmul(
                        sc_ps[:, 0 : t1 - t0],
                        st["kT"][:, i * 128 : (i + 1) * 128],
                        st["qT"][:, t0:t1],
                        start=True,
                        stop=True,
                    )
                    nc.scalar.activation(
                        st["attnT"][:, i, t0:t1], sc_ps[:, 0 : t1 - t0], EXP
                    )
                    if t0 == i * 128:
                        # mask the diagonal block (strict upper triangle keeps)
                        nc.gpsimd.tensor_mul(
                            st["attnT"][:, i, i * 128 : (i + 1) * 128],
                            st["attnT"][:, i, i * 128 : (i + 1) * 128],
                            mask[:],
                        )

                def nd_one(j):
                    nd_ps = pss.tile([65, 128], F32, tag="small")
                    for i in range(j + 1):
                        nc.tensor.matmul(
                            nd_ps[:],
                            st["v_aug"][:, i, :],
                            st["attnT"][:, i, j * 128 : (j + 1) * 128],
                            start=(i == 0),
                            stop=(i == j),
                        )
                    # endgame eviction on ACT: DVE's queue holds a 658ns norm
                    # right here, which would stall the final out matmuls
                    nc.scalar.copy(st["ndT"][:, j : j + 1, :], nd_ps[:])

                def nd_pair(j):
                    # numerator+denominator for t-chunks j, j+1
                    nd_ps = ps.tile([65, 256], F32, tag="big")
                    for k in range(2):
                        for i in range(j + k + 1):
                            nc.tensor.matmul(
                                nd_ps[:, k * 128 : (k + 1) * 128],
                                st["v_aug"][:, i, :],
                                st["attnT"][:, i, (j + k) * 128 : (j + k + 1) * 128],
                                start=(i == 0),
                                stop=(i == j + k),
                            )
                    nc.vector.tensor_copy(st["ndT"][:, j : j + 2, :], nd_ps[:])

                def dT(j):
                    dt_ps = pss.tile([128, 1], F32, tag="small")
                    nc.tensor.matmul(dt_ps[:], st["ndT"][:, j, :], unit,
                                     start=True, stop=True)
                    nc.vector.reciprocal(st["recip"][:, j : j + 1], dt_ps[:])

                def out_half(j, h, eng):
                    o_ps = ps.tile([128, 512], F32, tag="big")
                    nc.tensor.matmul(
                        o_ps[:],
                        st["ndT"][:, j, :],
                        wp[:, h * 512 : (h + 1) * 512],
                        start=True,
                        stop=True,
                    )
                    dst = st["out_sb"][:, j, h * 512 : (h + 1) * 512]
                    r = st["recip"][:, j : j + 1]
                    if eng == "act":
                        nc.scalar.activation(dst, o_ps[:], COPY, scale=r)
                    else:
                        nc.vector.tensor_scalar_mul(dst, o_ps[:], r)

                def store_act(j0, j1, h0, h1):
                    orr = out_d.ap()[b].rearrange("(c p) e -> p c e", p=128)
                    nc.scalar.dma_start(
                        out=orr[:, j0:j1, h0:h1],
                        in_=st["out_sb"][:, j0:j1, h0:h1],
                    )

                def store(j0, j1, h0=0, h1=D):
                    # Stores ride the ACT ring (batch 0) / DVE ring (batch 1)
                    # so the SP ring stays loads-only: a store's dma_start
                    # sem-waits at its ring head for norm data, and on the SP
                    # ring that would block the next timing-loop iteration's
                    # x prefetch (head-of-line), serializing iterations.
                    orr = out_d.ap()[b].rearrange("(c p) e -> p c e", p=128)
                    nc.sync.dma_start(
                        out=orr[:, j0:j1, h0:h1],
                        in_=st["out_sb"][:, j0:j1, h0:h1],
                    )

                return dict(
                    qk_half=qk_half, qk_evict=qk_evict,
                    v_chain_evict=v_chain_evict, sc_init=sc_init,
                    sc_piece=sc_piece, store_act=store_act,
                    v_chain_start=v_chain_start, nd_pair=nd_pair, nd_one=nd_one, dT=dT,
                    out_half=out_half, store=store,
                )

            def v_rest(B_):
                """v chains 6..7, sequential (x fully resident by then)."""
                for t_ in (6, 7):
                    for c in range(NCH):
                        B_["v_chain_start"](t_, c)
                    B_["v_chain_evict"](t_)

            def front_proj(B_):
                """qk + v projections (the batch's x-bound phase)."""
                B_["sc_init"]()
                B_["qk_half"](0)
                B_["qk_half"](1)
                for c in range(NCH):
                    for t_ in range(6):
                        B_["v_chain_start"](t_, c)
                B_["qk_evict"](0)
                B_["qk_evict"](1)
                for t_ in range(6):
                    B_["v_chain_evict"](t_)
                v_rest(B_)

            def sc_group(B_, g):
                """scores for chunk pair (2g, 2g+1)."""
                for i in (2 * g, 2 * g + 1):
                    B_["sc_piece"](i, 0)
                    B_["sc_piece"](i, 1)

            def nd_group(B_, g):
                j = 2 * g
                B_["nd_pair"](j)
                B_["dT"](j)
                B_["dT"](j + 1)

            def nd_single(B_, j):
                B_["nd_one"](j)
                B_["dT"](j)

            def out_single(B_, j, e0, e1, halves=False):
                if halves:
                    B_["out_half"](j, 0, e0)
                    B_["store_act"](j, j + 1, 0, 512)
                    B_["out_half"](j, 1, e1)
                    B_["store_act"](j, j + 1, 512, D)
                else:
                    B_["out_half"](j, 0, e0)
                    B_["out_half"](j, 1, e1)
                    B_["store"](j, j + 1)

            def out_group(B_, g, ne):
                j = 2 * g
                if g < 3:
                    B_["out_half"](j, 0, next(ne))
                    B_["out_half"](j, 1, next(ne))
                    B_["out_half"](j + 1, 0, next(ne))
                    B_["out_half"](j + 1, 1, next(ne))
                    B_["store"](j, j + 2)
                else:
                    # tail: fastest engines (ACT/DVE) on the final halves;
                    # whole-chunk stores (SP.SEQ costs ~650ns per dma_start,
                    # which dominates the tail when stores are split)
                    B_["out_half"](6, 0, "act")
                    B_["out_half"](6, 1, "dve")
                    B_["store"](6, 7)
                    B_["out_half"](7, 0, "act")
                    B_["out_half"](7, 1, "dve")
                    B_["store"](7, 8)

            NORM_ENGINES = ["dve", "act", "dve", "act", "dve", "act",
                            "dve", "act", "dve", "act", "dve", "act",
                            "dve", "act", "dve", "act"]

            def head_pieces():
                xr0 = xT_d.ap()[0].rearrange("(c p) t -> p c t", p=128)
                nc.sync.dma_start(out=xs[0][:, 0:1, 0:512],
                                  in_=xr0[:, 0:1, 0:512])
                nc.sync.dma_start(out=xs[0][:, 0:1, 512:T],
                                  in_=xr0[:, 0:1, 512:T])
                xload(0, [(1, 3)])

            # prologue: first pieces for iteration 1 (in the timing loop the
            # body's tail prefetches them for the next iteration)
            head_pieces()
            INTERLEAVED = False

            def run_once(_=None):
                emit_warmups(N_WARMUP)
                if INTERLEAVED:
                    xload(0, [(2, 3)])
                    xload(1, [(0, 2)])
                    xload(0, [(3, 4)])
                    xload(1, [(2, 4)])
                    xload(0, [(4, 6)])
                    xload(1, [(4, 6)])
                    xload(0, [(6, 8)])
                    xload(1, [(6, 8)])
                else:
                    xload(0, [(3, 5), (5, 8)])
                    xload(1, [(0, 8)])
                B0 = make_batch(0)
                B1 = make_batch(1)
                ne0 = iter(NORM_ENGINES)
                ne1 = iter(NORM_ENGINES)
                front_proj(B0)
                # merged back phases; B1's scores run early so its exps
                # stream on ACT during B0's out phase (ACT paces the tail)
                sc_group(B0, 0)
                sc_group(B0, 1)
                nd_group(B0, 0)
                sc_group(B0, 2)
                B1["qk_half"](0)
                nd_group(B0, 1)
                out_group(B0, 0, ne0)
                B1["qk_half"](1)
                B1["qk_evict"](0)
                B1["qk_evict"](1)
                B1["sc_init"]()
                sc_group(B0, 3)
                nd_group(B0, 2)
                sc_group(B1, 0)
                out_group(B0, 1, ne0)
                for c in range(NCH):
                    B1["v_chain_start"](0, c)
                    B1["v_chain_start"](1, c)
                B1["v_chain_evict"](0)
                B1["v_chain_evict"](1)
                nd_group(B0, 3)
                sc_group(B1, 1)
                nd_group(B1, 0)
                out_group(B0, 2, ne0)
                for t_ in range(2, NCH):
                    for c in range(NCH):
                        B1["v_chain_start"](t_, c)
                    B1["v_chain_evict"](t_)
                sc_group(B1, 2)
                nd_group(B1, 1)
                out_group(B1, 0, ne1)
                out_group(B0, 3, ne0)
                sc_group(B1, 3)
                nd_group(B1, 2)
                out_group(B1, 1, ne1)
                nd_single(B1, 6)
                out_group(B1, 2, ne1)
                nd_single(B1, 7)
                if n_reps > 1:
                    head_pieces()
                out_single(B1, 6, "act", "dve")
                out_single(B1, 7, "act", "dve", halves=True)

            def run_once_sym(_=None):
                emit_warmups(N_WARMUP)
                xload(0, [(2, 3)])
                xload(1, [(0, 2)])
                xload(0, [(3, 4)])
                xload(1, [(2, 4)])
                xload(0, [(4, 6)])
                xload(1, [(4, 6)])
                xload(0, [(6, 8)])
                xload(1, [(6, 8)])
                B0 = make_batch(0)
                B1 = make_batch(1)
                ne0 = iter(NORM_ENGINES)
                ne1 = iter(NORM_ENGINES)
                front_proj(B0)
                B1["sc_init"]()
                B1["qk_half"](0)
                B1["qk_half"](1)
                for c in range(NCH):
                    for t_ in range(6):
                        B1["v_chain_start"](t_, c)
                B1["qk_evict"](0)
                B1["qk_evict"](1)
                for t_ in range(6):
                    B1["v_chain_evict"](t_)
                v_rest(B1)
                sc_group(B0, 0)
                sc_group(B1, 0)
                sc_group(B0, 1)
                sc_group(B1, 1)
                nd_group(B0, 0)
                nd_group(B1, 0)
                sc_group(B0, 2)
                sc_group(B1, 2)
                nd_group(B0, 1)
                nd_group(B1, 1)
                out_group(B0, 0, ne0)
                out_group(B1, 0, ne1)
                sc_group(B0, 3)
                sc_group(B1, 3)
                nd_group(B0, 2)
                nd_group(B1, 2)
                out_group(B0, 1, ne0)
                out_group(B1, 1, ne1)
                nd_single(B0, 6)
                nd_single(B1, 6)
                out_group(B0, 2, ne0)
                out_group(B1, 2, ne1)
                nd_single(B0, 7)
                nd_single(B1, 7)
                if n_reps > 1:
                    head_pieces()
                out_single(B0, 6, "act", "dve")
                out_single(B1, 6, "act", "dve")
                out_single(B0, 7, "act", "dve", halves=True)
                out_single(B1, 7, "act", "dve", halves=True)

            body = run_once_sym if INTERLEAVED else run_once
            if n_reps == 1:
                body()
            else:
                with tc.For_i(0, n_reps, 1):
                    body()

    nc.compile()
    return nc


def _prep_inputs(x, Wq, Wk, Wv, Wp, bp):
    x = np.asarray(x, np.float32)
    Wq = np.asarray(Wq, np.float32)
    Wk = np.asarray(Wk, np.float32)
    Wv = np.asarray(Wv, np.float32)
    Wp = np.asarray(Wp, np.float32)
    bp = np.asarray(bp, np.float32)

    # fold the H-way tile-concat into Wp, the 1/sqrt(hs) scale into Wq
    Wp_eff = Wp.reshape(H, HS, D).sum(0)
    wp_aug = np.concatenate([Wp_eff, bp[None, :]], 0).astype(BF_NP)  # [65, 1024]
    wqk = np.concatenate([Wq * np.float32(1.0 / np.sqrt(HS)), Wk], 1).astype(BF_NP)
    wv = Wv.astype(BF_NP)
    xT = np.ascontiguousarray(x.transpose(0, 2, 1)).astype(X_DT_NP)

    mask = np.triu(np.ones((128, 128), np.float32)).astype(BF_NP)
    idb = np.eye(64, dtype=np.float32).astype(BF_NP)

    # packed constants [128, CONST_W]
    cpack = np.zeros((128, CONST_W), BF_NP)
    cpack[:, OFF_WQK : OFF_WQK + 1024] = (
        wqk.reshape(NCH, 128, 128).transpose(1, 0, 2).reshape(128, 1024)
    )
    cpack[:, OFF_WV : OFF_WV + 512] = (
        wv.reshape(NCH, 128, 64).transpose(1, 0, 2).reshape(128, 512)
    )
    cpack[0:65, OFF_WP : OFF_WP + 1024] = wp_aug
    cpack[:, OFF_MASK : OFF_MASK + 128] = mask
    cpack[0:64, OFF_IDB : OFF_IDB + 64] = idb
    cpack[64, OFF_UNIT] = np.float32(1.0)  # unit65 row 64

    in_maps = []
    for c in range(NCORES):
        in_maps.append(
            {
                "xT": np.ascontiguousarray(xT[c * NB : (c + 1) * NB]),
                "cpack": cpack,
            }
        )
    return in_maps


_NC_CACHE = {}


def kernel(x, Wq, Wk, Wv, Wp, bp):
    in_maps = _prep_inputs(x, Wq, Wk, Wv, Wp, bp)
    if "nc" not in _NC_CACHE:
        _NC_CACHE["nc"] = _build_nc(n_reps=1)
    nc = _NC_CACHE["nc"]
    last_err = None
    for _ in range(3):  # retry: the axon transport occasionally hiccups
        try:
            res = run_bass_kernel_spmd(nc, in_maps, core_ids=list(range(NCORES)))
            out = np.concatenate([np.asarray(r["out"]) for r in res.results], 0)
            return np.ascontiguousarray(out.astype(np.float32))
        except Exception as e:  # noqa: BLE001
            last_err = e
    raise last_err



# revision 33
# speedup vs baseline: 1.3019x; 1.3019x over previous
"""Trainium2 Bass kernel v2 for nn_MultiHeadAttention_85925115723936.

Contract: kernel(**inputs) takes the FULL unsharded inputs from
setup_inputs() (x [16,1024,1024] f32, Wq/Wk/Wv [1024,64], Wp [1024,1024],
bp [1024]) and returns the FULL [16, 1024, 1024] float32 output.

Sharding: data-parallel over batch — 16 batches across 8 NeuronCores
(2 per core), zero cross-core communication.

All H=16 heads share one weight set, so the H-way concat collapses into a
folded projection (wp_eff = sum_h Wp[h*hs:(h+1)*hs, :]), bias folded as an
extra row driven by the softmax denominator; only the causal triangle of
scores is computed; the denominator falls out of a ones-column in v_aug.

x is loaded as float8_e3m4 (e3m4's 4 mantissa bits keep l2 error at
~1.4% vs the 2e-2 gate while halving input DMA; the PE consumes mixed
fp8/bf16 operands directly).

v2 structural changes vs v1:
- PE warmup matmuls on a zeroed scratch tile cover the initial x-load fill
  so the PE p-state ramp (0.65->1.2->2.4GHz after 3us continuous busy)
  completes before real work starts, and the PE never idles at the head.
- All constants arrive in ONE packed [128, 2753] DMA (HWDGE descriptor
  generation is a serialized ~630ns/DMA shared resource): wqk | wv | wp_aug
  | mask | idb | unit.
- PSUM is managed as a single rotating ring of 1-bank [128,512]-shaped
  tiles (bufs=7) + 1 small bank, so qk/v/scores/nd/out all timeshare the
  8 banks without static carve-out conflicts.
- Scores matmuls are causally trimmed (only t >= s-chunk start computed).
- Evictions/normalizations are spread across ACT/DVE/Pool; exp stays on ACT.
- Batch b+1's qk/v matmuls are interleaved into batch b's nd/out phase to
  keep the PE gapless.
"""

import numpy as np
import ml_dtypes

import concourse.mybir as mybir
import concourse.tile as tile
from concourse import bacc
from concourse.bass_utils import run_bass_kernel_spmd

BF_NP = ml_dtypes.bfloat16
BF = mybir.dt.bfloat16
F32 = mybir.dt.float32
EXP = mybir.ActivationFunctionType.Exp
COPY = mybir.ActivationFunctionType.Copy

B, T, D, H, HS = 16, 1024, 1024, 16, 64
NCORES = 8
NB = B // NCORES     # batches per core
NCH = 8              # 1024 / 128 chunks
OUT_DT = BF

# packed-constant free-dim offsets (bf16 elems per partition)
OFF_WQK = 0          # [128, 8, 128] -> 1024
OFF_WV = 1024        # [128, 8, 64]  -> 512
OFF_WP = 1536        # [65, 1024] on partitions 0:65
OFF_MASK = 2560      # [128, 128]
OFF_IDB = 2688       # [64, 64] on partitions 0:64
OFF_UNIT = 2752      # [65, 1] on partitions 0:65
CONST_W = 2753

N_WARMUP = 0
UNROLL_REPS = False  # sim-only: unroll the timing loop instead of For_i
X_DT = mybir.dt.float8e3
X_DT_NP = ml_dtypes.float8_e3m4



def _build_nc(n_reps: int = 1, n_cores: int = NCORES):
    # Warmup matmuls ramp the PE p-state at cold start; in an n_reps>1
    # timing loop the PE is already hot from the previous iteration, so
    # they would be pure per-iteration overhead.
    warmup = N_WARMUP > 0
    nc = bacc.Bacc("TRN2", target_bir_lowering=False, debug=False, num_devices=n_cores)

    xT_d = nc.dram_tensor("xT", [NB, D, T], X_DT, kind="ExternalInput")
    cpack_d = nc.dram_tensor("cpack", [128, CONST_W], BF, kind="ExternalInput")
    out_d = nc.dram_tensor("out", [NB, T, D], OUT_DT, kind="ExternalOutput")

    with tile.TileContext(nc) as tc:
        with (
            tc.tile_pool(name="const", bufs=1) as cpool,
            tc.tile_pool(name="sb", bufs=2) as sb,
            tc.tile_pool(name="ps", bufs=6, space="PSUM") as ps,
            tc.tile_pool(name="pss", bufs=2, space="PSUM") as pss,
        ):
            # ---- constants: one packed DMA on the ACT ring; wqk columns
            # first so the first qk matmul's weights land earliest ----
            cs = cpool.tile([128, CONST_W], BF, tag="cpack")
            nc.scalar.dma_start(out=cs[:, 0:1024], in_=cpack_d.ap()[:, 0:1024])

            def wqk(c):
                return cs[:, OFF_WQK + c * 128 : OFF_WQK + (c + 1) * 128]

            def wv(c):
                return cs[:, OFF_WV + c * 64 : OFF_WV + (c + 1) * 64]

            wp = cs[0:65, OFF_WP : OFF_WP + 1024]
            mask = cs[:, OFF_MASK : OFF_MASK + 128]
            idb = cs[0:64, OFF_IDB : OFF_IDB + 64]
            unit = cs[0:65, OFF_UNIT : OFF_UNIT + 1]

            # ---- warmup scratch (zeroed so CoreSim finiteness holds) ----
            scratch = cpool.tile([128, 512], BF, tag="scratch")
            nc.vector.memset(scratch[:], 0.0)

            # ---- x loads: all emitted upfront on the SP ring; batch0 in
            # fine grains so qk/v are never head-of-line blocked ----
            xs = []
            for b in range(NB):
                xt = sb.tile([128, NCH, T], X_DT, tag=f"x{b}", bufs=1, name=f"x{b}")
                xs.append(xt)

            def xload(b, pieces):
                xr = xT_d.ap()[b].rearrange("(c p) t -> p c t", p=128)
                for c0, c1 in pieces:
                    nc.sync.dma_start(out=xs[b][:, c0:c1, :], in_=xr[:, c0:c1, :])

            # cpack part 2 (wv/wp/mask/...) emitted directly after part 1;
            # x loads are emitted per iteration inside run_once
            nc.scalar.dma_start(
                out=cs[:, 1024:CONST_W], in_=cpack_d.ap()[:, 1024:CONST_W]
            )

            # ---- warmup matmuls: fill the DMA window, ramp the PE clock ----
            def emit_warmups(n):
                for w in range(n):
                    wu = ps.tile([128, 512], F32, tag="big", name="wu")
                    nc.tensor.matmul(wu[:], scratch[:, 0:128], scratch[:],
                                     start=True, stop=True)

            # ---------------- per-batch emission pieces ----------------
            # Each batch body is split into closures so batch b+1's qk/v can
            # be interleaved into batch b's nd/out phase.

            def make_batch(b):
                st = {}
                xt = xs[b]

                def xsl(c, h):
                    return xt[:, c, h * 512 : (h + 1) * 512]

                def qk_half(h):
                    qk_ps = ps.tile([128, 512], F32, tag="big")
                    for c in range(NCH):
                        nc.tensor.matmul(
                            qk_ps[:],
                            wqk(c),
                            xsl(c, h),
                            start=(c == 0),
                            stop=(c == NCH - 1),
                        )
                    st[f"qk{h}"] = qk_ps

                def qk_evict(h):
                    if "qT" not in st:
                        st["qT"] = sb.tile([64, T], BF, tag="qT", name="qT")
                        st["kT"] = sb.tile([64, T], BF, tag="kT", name="kT")
                    qp = st[f"qk{h}"]
                    nc.vector.tensor_copy(
                        st["qT"][:, h * 512 : (h + 1) * 512], qp[0:64, :]
                    )
                    nc.vector.tensor_copy(
                        st["kT"][:, h * 512 : (h + 1) * 512], qp[64:128, :]
                    )

                def v_chain_start(t_, c):
                    # direct-layout v for t-chunk t_: accumulate over
                    # d-chunks with x as the stationary operand -> v lands
                    # as [t, h] (v_aug orientation), no transposes. Chains
                    # run t-outer (x is fully prefetched), ping-ponging a
                    # dedicated 2-bank ring.
                    key = f"v_ps{t_}"
                    if key not in st:
                        st[key] = pss.tile([128, HS], F32, tag="small", name=key)
                    nc.tensor.matmul(
                        st[key][:],
                        xt[:, c, t_ * 128 : (t_ + 1) * 128],
                        wv(c),
                        start=(c == 0),
                        stop=(c == NCH - 1),
                    )

                def v_chain_evict(t_):
                    nc.vector.tensor_copy(
                        st["v_aug"][:, t_, 0:HS], st[f"v_ps{t_}"][:]
                    )
                    del st[f"v_ps{t_}"]

                def sc_init():
                    st["attnT"] = sb.tile([128, NCH, T], BF, tag="attnT", name="attnT")
                    st["v_aug"] = sb.tile([128, NCH, HS + 1], BF, tag="v_aug", name="v_aug")
                    nc.gpsimd.memset(st["v_aug"][:, :, HS], 1.0)
                    st["ndT"] = sb.tile([65, NCH, 128], BF, tag="ndT", name="ndT")
                    st["recip"] = sb.tile([128, NCH], F32, tag="recip", name="recip")
                    st["out_sb"] = sb.tile([128, NCH, D], OUT_DT, tag="out_sb", name="out_sb")

                def sc_piece(i, h):
                    # scores^T for s-chunk i, t in [max(128i, 512h), 512(h+1))
                    t0 = max(i * 128, h * 512)
                    t1 = (h + 1) * 512
                    if t0 >= t1:
                        return
                    sc_ps = ps.tile([128, 512], F32, tag="big")
                    nc.tensor.matmul(
                        sc_ps[:, 0 : t1 - t0],
                        st["kT"][:, i * 128 : (i + 1) * 128],
                        st["qT"][:, t0:t1],
                        start=True,
                        stop=True,
                    )
                    nc.scalar.activation(
                        st["attnT"][:, i, t0:t1], sc_ps[:, 0 : t1 - t0], EXP
                    )
                    if t0 == i * 128:
                        # mask the diagonal block (strict upper triangle keeps)
                        nc.gpsimd.tensor_mul(
                            st["attnT"][:, i, i * 128 : (i + 1) * 128],
                            st["attnT"][:, i, i * 128 : (i + 1) * 128],
                            mask[:],
                        )

                def nd_one(j):
                    nd_ps = pss.tile([65, 128], F32, tag="small")
                    for i in range(j + 1):
                        nc.tensor.matmul(
                            nd_ps[:],
                            st["v_aug"][:, i, :],
                            st["attnT"][:, i, j * 128 : (j + 1) * 128],
                            start=(i == 0),
                            stop=(i == j),
                        )
                    # endgame eviction on ACT: DVE's queue holds a 658ns norm
                    # right here, which would stall the final out matmuls
                    nc.scalar.copy(st["ndT"][:, j : j + 1, :], nd_ps[:])

                def nd_pair(j):
                    # numerator+denominator for t-chunks j, j+1; chunk j's
                    # eviction is emitted between the two chains so ndT(j)
                    # (the out/dT stationary) lands as early as possible
                    nd_ps = ps.tile([65, 256], F32, tag="big")
                    for k in range(2):
                        for i in range(j + k + 1):
                            nc.tensor.matmul(
                                nd_ps[:, k * 128 : (k + 1) * 128],
                                st["v_aug"][:, i, :],
                                st["attnT"][:, i, (j + k) * 128 : (j + k + 1) * 128],
                                start=(i == 0),
                                stop=(i == j + k),
                            )
                    nc.vector.tensor_copy(st["ndT"][:, j : j + 2, :], nd_ps[:])

                def dT(j):
                    dt_ps = pss.tile([128, 1], F32, tag="small")
                    nc.tensor.matmul(dt_ps[:], st["ndT"][:, j, :], unit,
                                     start=True, stop=True)
                    nc.vector.reciprocal(st["recip"][:, j : j + 1], dt_ps[:])

                def dT_pair(j):
                    # both denominators of an nd pair -> one [128,2] tile,
                    # one reciprocal (halves the tiny-op count on PE/DVE)
                    dt_ps = pss.tile([128, 2], F32, tag="small")
                    nc.tensor.matmul(dt_ps[:, 0:1], st["ndT"][:, j, :], unit,
                                     start=True, stop=True)
                    nc.tensor.matmul(dt_ps[:, 1:2], st["ndT"][:, j + 1, :],
                                     unit, start=True, stop=True)
                    nc.vector.reciprocal(st["recip"][:, j : j + 2], dt_ps[:])

                def out_half(j, h, eng):
                    o_ps = ps.tile([128, 512], F32, tag="big")
                    nc.tensor.matmul(
                        o_ps[:],
                        st["ndT"][:, j, :],
                        wp[:, h * 512 : (h + 1) * 512],
                        start=True,
                        stop=True,
                    )
                    dst = st["out_sb"][:, j, h * 512 : (h + 1) * 512]
                    r = st["recip"][:, j : j + 1]
                    if eng == "act":
                        nc.scalar.activation(dst, o_ps[:], COPY, scale=r)
                    elif eng == "pool":
                        nc.gpsimd.tensor_scalar_mul(dst, o_ps[:], r)
                    else:
                        nc.vector.tensor_scalar_mul(dst, o_ps[:], r)

                def store_act(j0, j1, h0, h1):
                    orr = out_d.ap()[b].rearrange("(c p) e -> p c e", p=128)
                    nc.scalar.dma_start(
                        out=orr[:, j0:j1, h0:h1],
                        in_=st["out_sb"][:, j0:j1, h0:h1],
                    )

                def store(j0, j1, h0=0, h1=D):
                    # Stores ride the ACT ring (batch 0) / DVE ring (batch 1)
                    # so the SP ring stays loads-only: a store's dma_start
                    # sem-waits at its ring head for norm data, and on the SP
                    # ring that would block the next timing-loop iteration's
                    # x prefetch (head-of-line), serializing iterations.
                    orr = out_d.ap()[b].rearrange("(c p) e -> p c e", p=128)
                    nc.sync.dma_start(
                        out=orr[:, j0:j1, h0:h1],
                        in_=st["out_sb"][:, j0:j1, h0:h1],
                    )

                return dict(
                    qk_half=qk_half, qk_evict=qk_evict,
                    v_chain_evict=v_chain_evict, sc_init=sc_init,
                    sc_piece=sc_piece, store_act=store_act,
                    v_chain_start=v_chain_start, nd_pair=nd_pair, nd_one=nd_one, dT=dT, dT_pair=dT_pair,
                    out_half=out_half, store=store,
                )

            def front_proj(B_, early_sc=True):
                """qk + v projections (the batch's x-bound phase).

                v chains run t-outer through the dedicated 2-bank psv ring;
                score pieces for groups 0-1 are interleaved between chains so
                ACT's exp stream starts during the v phase (exp lookahead
                for the nd matmuls of the back phase)."""
                B_["sc_init"]()
                B_["qk_half"](0)
                B_["qk_half"](1)
                B_["qk_evict"](0)
                B_["qk_evict"](1)
                # pieces (i, h) for i<=3 (groups 0-1); h=0 first (earliest
                # eviction dependency)
                pieces = [(0, 0), (1, 0), (2, 0), (3, 0),
                          (0, 1), (1, 1), (2, 1), (3, 1)]
                for t_ in range(NCH):
                    for c in range(NCH):
                        B_["v_chain_start"](t_, c)
                    B_["v_chain_evict"](t_)
                    if early_sc and t_ % 2 == 1:
                        i, h = pieces[t_ // 2]
                        B_["sc_piece"](i, h)
                if early_sc:
                    for i, h in pieces[4:]:
                        B_["sc_piece"](i, h)

            def sc_group(B_, g):
                """scores for chunk pair (2g, 2g+1)."""
                for i in (2 * g, 2 * g + 1):
                    B_["sc_piece"](i, 0)
                    B_["sc_piece"](i, 1)

            def nd_group(B_, g):
                j = 2 * g
                B_["nd_pair"](j)
                B_["dT_pair"](j)

            def nd_single(B_, j):
                B_["nd_one"](j)
                B_["dT"](j)

            def out_single(B_, j, e0, e1, halves=False):
                if halves:
                    B_["out_half"](j, 0, e0)
                    B_["store_act"](j, j + 1, 0, 512)
                    B_["out_half"](j, 1, e1)
                    B_["store_act"](j, j + 1, 512, D)
                else:
                    B_["out_half"](j, 0, e0)
                    B_["out_half"](j, 1, e1)
                    B_["store"](j, j + 1)

            def out_group(B_, g, ne):
                j = 2 * g
                if g < 3:
                    B_["out_half"](j, 0, next(ne))
                    B_["out_half"](j, 1, next(ne))
                    B_["out_half"](j + 1, 0, next(ne))
                    B_["out_half"](j + 1, 1, next(ne))
                    B_["store"](j, j + 2)
                else:
                    # tail: fastest engines (ACT/DVE) on the final halves;
                    # whole-chunk stores (SP.SEQ costs ~650ns per dma_start,
                    # which dominates the tail when stores are split)
                    B_["out_half"](6, 0, "act")
                    B_["out_half"](6, 1, "dve")
                    B_["store"](6, 7)
                    B_["out_half"](7, 0, "act")
                    B_["out_half"](7, 1, "dve")
                    B_["store"](7, 8)

            # NOTE: GPSIMD/Pool cannot access PSUM on HW (BIR verifier
            # rejects it), so out-scale evictions may only use dve/act.
            NORM_ENGINES = ["dve", "act", "dve", "act", "dve", "act",
                            "dve", "act", "dve", "act", "dve", "act",
                            "dve", "act", "dve", "act"]

            def head_pieces():
                xr0 = xT_d.ap()[0].rearrange("(c p) t -> p c t", p=128)
                nc.sync.dma_start(out=xs[0][:, 0:1, 0:512],
                                  in_=xr0[:, 0:1, 0:512])
                nc.sync.dma_start(out=xs[0][:, 0:1, 512:T],
                                  in_=xr0[:, 0:1, 512:T])
                xload(0, [(1, 3), (3, 8)])
                xload(1, [(0, 4), (4, 8)])

            # prologue: full x for iteration 1 (in the timing loop the body
            # mid-phases prefetch the next iteration's x after last use)
            head_pieces()
            INTERLEAVED = False

            def run_once(_=None):
                emit_warmups(N_WARMUP)
                B0 = make_batch(0)
                B1 = make_batch(1)
                ne0 = iter(NORM_ENGINES)
                ne1 = iter(NORM_ENGINES)
                front_proj(B0)
                # b0's x fully consumed by front_proj: prefetch next iter now
                if n_reps > 1:
                    xload(0, [(0, 4), (4, 8)])
                # back phases; B1's front interleaves into B0's back so the
                # PE stays fed while B0's evictions/exps drain
                nd_group(B0, 0)
                sc_group(B0, 2)
                nd_group(B0, 1)
                B1["qk_half"](0)
                out_group(B0, 0, ne0)
                B1["qk_half"](1)
                B1["qk_evict"](0)
                B1["qk_evict"](1)
                B1["sc_init"]()
                sc_group(B0, 3)
                nd_group(B0, 2)
                out_group(B0, 1, ne0)
                # B1 v phase (t-outer) with B1's early scores interleaved
                pieces1 = [(0, 0), (1, 0), (2, 0), (3, 0),
                           (0, 1), (1, 1), (2, 1), (3, 1)]
                for t_ in range(NCH):
                    for c in range(NCH):
                        B1["v_chain_start"](t_, c)
                    B1["v_chain_evict"](t_)
                    if t_ % 2 == 1:
                        i, h = pieces1[t_ // 2]
                        B1["sc_piece"](i, h)
                # b1's x fully consumed: prefetch next iter
                if n_reps > 1:
                    xload(1, [(0, 4), (4, 8)])
                nd_group(B0, 3)
                for i, h in pieces1[4:]:
                    B1["sc_piece"](i, h)
                out_group(B0, 2, ne0)
                nd_group(B1, 0)
                sc_group(B1, 2)
                out_group(B0, 3, ne0)
                nd_group(B1, 1)
                out_group(B1, 0, ne1)
                sc_group(B1, 3)
                nd_group(B1, 2)
                out_group(B1, 1, ne1)
                nd_single(B1, 6)
                out_group(B1, 2, ne1)
                nd_single(B1, 7)
                out_single(B1, 6, "act", "dve")
                out_single(B1, 7, "act", "dve", halves=True)

            def run_once_sym(_=None):
                emit_warmups(N_WARMUP)
                xload(0, [(2, 3)])
                xload(1, [(0, 2)])
                xload(0, [(3, 4)])
                xload(1, [(2, 4)])
                xload(0, [(4, 6)])
                xload(1, [(4, 6)])
                xload(0, [(6, 8)])
                xload(1, [(6, 8)])
                B0 = make_batch(0)
                B1 = make_batch(1)
                ne0 = iter(NORM_ENGINES)
                ne1 = iter(NORM_ENGINES)
                front_proj(B0)
                B1["sc_init"]()
                B1["qk_half"](0)
                B1["qk_half"](1)
                for c in range(NCH):
                    for t_ in range(6):
                        B1["v_chain_start"](t_, c)
                B1["qk_evict"](0)
                B1["qk_evict"](1)
                for t_ in range(6):
                    B1["v_chain_evict"](t_)
                v_rest(B1)
                sc_group(B0, 0)
                sc_group(B1, 0)
                sc_group(B0, 1)
                sc_group(B1, 1)
                nd_group(B0, 0)
                nd_group(B1, 0)
                sc_group(B0, 2)
                sc_group(B1, 2)
                nd_group(B0, 1)
                nd_group(B1, 1)
                out_group(B0, 0, ne0)
                out_group(B1, 0, ne1)
                sc_group(B0, 3)
                sc_group(B1, 3)
                nd_group(B0, 2)
                nd_group(B1, 2)
                out_group(B0, 1, ne0)
                out_group(B1, 1, ne1)
                nd_single(B0, 6)
                nd_single(B1, 6)
                out_group(B0, 2, ne0)
                out_group(B1, 2, ne1)
                nd_single(B0, 7)
                nd_single(B1, 7)
                if n_reps > 1:
                    head_pieces()
                out_single(B0, 6, "act", "dve")
                out_single(B1, 6, "act", "dve")
                out_single(B0, 7, "act", "dve", halves=True)
                out_single(B1, 7, "act", "dve", halves=True)

            body = run_once_sym if INTERLEAVED else run_once
            if n_reps == 1:
                body()
            elif UNROLL_REPS:
                for _ in range(n_reps):
                    body()
            else:
                with tc.For_i(0, n_reps, 1):
                    body()

    nc.compile()
    return nc


def _prep_inputs(x, Wq, Wk, Wv, Wp, bp):
    x = np.asarray(x, np.float32)
    Wq = np.asarray(Wq, np.float32)
    Wk = np.asarray(Wk, np.float32)
    Wv = np.asarray(Wv, np.float32)
    Wp = np.asarray(Wp, np.float32)
    bp = np.asarray(bp, np.float32)

    # fold the H-way tile-concat into Wp, the 1/sqrt(hs) scale into Wq
    Wp_eff = Wp.reshape(H, HS, D).sum(0)
    wp_aug = np.concatenate([Wp_eff, bp[None, :]], 0).astype(BF_NP)  # [65, 1024]
    wqk = np.concatenate([Wq * np.float32(1.0 / np.sqrt(HS)), Wk], 1).astype(BF_NP)
    wv = Wv.astype(BF_NP)
    xT = np.ascontiguousarray(x.transpose(0, 2, 1)).astype(X_DT_NP)

    mask = np.triu(np.ones((128, 128), np.float32)).astype(BF_NP)
    idb = np.eye(64, dtype=np.float32).astype(BF_NP)

    # packed constants [128, CONST_W]
    cpack = np.zeros((128, CONST_W), BF_NP)
    cpack[:, OFF_WQK : OFF_WQK + 1024] = (
        wqk.reshape(NCH, 128, 128).transpose(1, 0, 2).reshape(128, 1024)
    )
    cpack[:, OFF_WV : OFF_WV + 512] = (
        wv.reshape(NCH, 128, 64).transpose(1, 0, 2).reshape(128, 512)
    )
    cpack[0:65, OFF_WP : OFF_WP + 1024] = wp_aug
    cpack[:, OFF_MASK : OFF_MASK + 128] = mask
    cpack[0:64, OFF_IDB : OFF_IDB + 64] = idb
    cpack[64, OFF_UNIT] = np.float32(1.0)  # unit65 row 64

    in_maps = []
    for c in range(NCORES):
        in_maps.append(
            {
                "xT": np.ascontiguousarray(xT[c * NB : (c + 1) * NB]),
                "cpack": cpack,
            }
        )
    return in_maps


_NC_CACHE = {}


def kernel(x, Wq, Wk, Wv, Wp, bp):
    in_maps = _prep_inputs(x, Wq, Wk, Wv, Wp, bp)
    if "nc" not in _NC_CACHE:
        _NC_CACHE["nc"] = _build_nc(n_reps=1)
    nc = _NC_CACHE["nc"]
    last_err = None
    for _ in range(3):  # retry: the axon transport occasionally hiccups
        try:
            res = run_bass_kernel_spmd(nc, in_maps, core_ids=list(range(NCORES)))
            out = np.concatenate([np.asarray(r["out"]) for r in res.results], 0)
            return np.ascontiguousarray(out.astype(np.float32))
        except Exception as e:  # noqa: BLE001
            last_err = e
    raise last_err

